# revision 7
# baseline (speedup 1.0000x reference)
"""ChunkRanker Bass kernel for Trainium2, 8-core data-parallel.

Math per chunk n (chunks: [4096, 128, 64] f32):
  flat = chunks[n].reshape(8192)
  std  = std(flat, ddof=1)
  realism = std<0.01 ? 10*std : (std>0.5 ? 0.5/std : 1-|std-0.1|)
  ctx    = previous_context[-10:].flatten()            # [640]
  starts = flat[:640]
  boundary = dot(starts, ctx) / max(|starts|*|ctx|, 1e-8)
  score = realism + 0.15 + 0.2*boundary

Sharding: leading chunk axis split 8 ways (512 chunks/core); ctx broadcast.
Per-core layout: 4 tiles of [128 partitions = chunks, 8192 free = elements],
each tile a single contiguous 4 MB HBM->SBUF DMA.
Per tile: DVE does sum (tensor_reduce) and dot-with-ctx (tensor_tensor_reduce);
ACT does sum-of-squares and sum-of-squares-of-starts (Square + accum_out).
The scalar tail (std, piecewise realism, cosine denom) runs once on [128, 4].
"""

import numpy as np

import concourse.bacc as bacc
import concourse.bass as bass
import concourse.mybir as mybir
import concourse.tile as tile
from concourse.bass_utils import run_bass_kernel_spmd

N_CORES = 8
N_TOTAL = 4096
N_LOC = N_TOTAL // N_CORES  # 512 chunks per core
P = 128                     # chunks per tile (partition dim)
T = N_LOC // P              # 4 tiles per core
D = 128 * 64                # 8192 elements per chunk
S = 10 * 64                 # 640 boundary elements
EPS = 1e-8

F32 = mybir.dt.float32
ALU = mybir.AluOpType
ACTF = mybir.ActivationFunctionType


def _build() -> bass.Bass:
    nc = bacc.Bacc(
        "TRN2", target_bir_lowering=False, debug=False, num_devices=N_CORES
    )
    x = nc.dram_tensor("chunks", [N_LOC, 128, 64], F32, kind="ExternalInput")
    ctx_in = nc.dram_tensor("ctx", [S], F32, kind="ExternalInput")
    out = nc.dram_tensor("out", [P, T], F32, kind="ExternalOutput")

    xf = x[:].rearrange("(t p) r f -> t p (r f)", p=P)  # [T, 128, 8192]

    with tile.TileContext(nc) as tc:
        with (
            tc.tile_pool(name="main", bufs=3) as main,
            tc.tile_pool(name="scratch", bufs=1) as scratch,
            tc.tile_pool(name="small", bufs=1) as small,
        ):
            # ctx broadcast to all 128 partitions (one small SWDGE DMA)
            ctxb = small.tile([P, S], F32)
            cap = ctx_in[:]
            nc.gpsimd.dma_start(
                out=ctxb,
                in_=bass.AP(tensor=cap.tensor, offset=cap.offset, ap=[[0, P], *cap.ap]),
            )

            sums = small.tile([P, T], F32)
            sumsqs = small.tile([P, T], F32)
            nums = small.tile([P, T], F32)
            startsqs = small.tile([P, T], F32)

            # |ctx|^2, identical value on every partition
            # (scalar_tensor_tensor: out = (in0*1)*in1, accum_out = sum(out);
            #  tensor_tensor_reduce would be one op too, but it faults trn2 here)
            cn2 = small.tile([P, 1], F32)
            scr_ttr = scratch.tile([P, S], F32, tag="scr_ttr")
            nc.vector.scalar_tensor_tensor(
                out=scr_ttr, in0=ctxb, scalar=1.0, in1=ctxb,
                op0=ALU.mult, op1=ALU.mult, accum_out=cn2,
            )

            for t in range(T):
                xt = main.tile([P, D], F32, tag="xt")
                nc.sync.dma_start(out=xt, in_=xf[t])
                # DVE: per-chunk sum over all 8192 elements
                nc.vector.tensor_reduce(
                    out=sums[:, t : t + 1], in_=xt,
                    axis=mybir.AxisListType.X, op=ALU.add,
                )
                # ACT: per-chunk sum of squares (Square writes scratch, accum is the result)
                scr_act = scratch.tile([P, D], F32, tag="scr_act")
                nc.scalar.activation(
                    out=scr_act, in_=xt, func=ACTF.Square,
                    accum_out=sumsqs[:, t : t + 1],
                )
                # DVE: dot(starts, ctx) per chunk
                scr_n = scratch.tile([P, S], F32, tag="scr_ttr")
                nc.vector.scalar_tensor_tensor(
                    out=scr_n, in0=xt[:, :S], scalar=1.0, in1=ctxb,
                    op0=ALU.mult, op1=ALU.mult, accum_out=nums[:, t : t + 1],
                )
                # ACT: |starts|^2 per chunk
                scr_sa = scratch.tile([P, S], F32, tag="scr_act_s")
                nc.scalar.activation(
                    out=scr_sa, in_=xt[:, :S], func=ACTF.Square,
                    accum_out=startsqs[:, t : t + 1],
                )

            # ---- tail on [128, T] ----
            # var*(D-1) = sumsq - sum^2/D ; std = sqrt(v1 / (D-1))
            t0 = small.tile([P, T], F32)
            nc.vector.scalar_tensor_tensor(
                out=t0, in0=sums, scalar=1.0 / D, in1=sums,
                op0=ALU.mult, op1=ALU.mult,
            )
            v1 = small.tile([P, T], F32)
            nc.vector.tensor_tensor(out=v1, in0=sumsqs, in1=t0, op=ALU.subtract)
            std = small.tile([P, T], F32)
            nc.scalar.activation(
                out=std, in_=v1, func=ACTF.Sqrt, scale=1.0 / (D - 1),
            )

            # piecewise realism (+0.15 regime term folded into each branch)
            b1 = small.tile([P, T], F32)
            nc.vector.tensor_scalar(
                out=b1, in0=std, scalar1=10.0, scalar2=0.15,
                op0=ALU.mult, op1=ALU.add,
            )
            rec = small.tile([P, T], F32)
            nc.vector.reciprocal(out=rec, in_=std)
            b2 = small.tile([P, T], F32)
            nc.vector.tensor_scalar(
                out=b2, in0=rec, scalar1=0.5, scalar2=0.15,
                op0=ALU.mult, op1=ALU.add,
            )
            d1 = small.tile([P, T], F32)
            nc.vector.tensor_scalar(
                out=d1, in0=std, scalar1=0.1, scalar2=None, op0=ALU.subtract,
            )
            aab = small.tile([P, T], F32)
            nc.vector.scalar_tensor_tensor(
                out=aab, in0=d1, scalar=-1.0, in1=d1, op0=ALU.mult, op1=ALU.max,
            )
            b3 = small.tile([P, T], F32)
            nc.vector.tensor_scalar(
                out=b3, in0=aab, scalar1=-1.0, scalar2=1.15,
                op0=ALU.mult, op1=ALU.add,
            )
            m1 = small.tile([P, T], mybir.dt.uint8)
            nc.vector.tensor_scalar(
                out=m1, in0=std, scalar1=0.01, scalar2=None, op0=ALU.is_lt,
            )
            m2 = small.tile([P, T], mybir.dt.uint8)
            nc.vector.tensor_scalar(
                out=m2, in0=std, scalar1=0.5, scalar2=None, op0=ALU.is_gt,
            )
            r1 = small.tile([P, T], F32)
            nc.vector.select(out=r1, mask=m2, on_true=b2, on_false=b3)
            realism = small.tile([P, T], F32)
            nc.vector.select(out=realism, mask=m1, on_true=b1, on_false=r1)

            # boundary = num / max(sqrt(startsq * |ctx|^2), eps)
            d2 = small.tile([P, T], F32)
            nc.vector.tensor_scalar(
                out=d2, in0=startsqs, scalar1=cn2, scalar2=None, op0=ALU.mult,
            )
            den = small.tile([P, T], F32)
            nc.scalar.activation(out=den, in_=d2, func=ACTF.Sqrt)
            den2 = small.tile([P, T], F32)
            nc.vector.tensor_scalar(
                out=den2, in0=den, scalar1=EPS, scalar2=None, op0=ALU.max,
            )
            rden = small.tile([P, T], F32)
            nc.vector.reciprocal(out=rden, in_=den2)
            bnd = small.tile([P, T], F32)
            nc.vector.tensor_tensor(out=bnd, in0=nums, in1=rden, op=ALU.mult)

            final = small.tile([P, T], F32)
            nc.vector.scalar_tensor_tensor(
                out=final, in0=bnd, scalar=0.2, in1=realism,
                op0=ALU.mult, op1=ALU.add,
            )
            nc.sync.dma_start(out=out[:], in_=final)
    nc.compile()
    return nc


_NC_CACHE = None


def _get_nc() -> bass.Bass:
    global _NC_CACHE
    if _NC_CACHE is None:
        _NC_CACHE = _build()
    return _NC_CACHE


def run(inputs: dict, trace: bool = False, **kw):
    """Returns (output [4096] f32, BassKernelResults)."""
    chunks = np.ascontiguousarray(np.asarray(inputs["chunks"], dtype=np.float32))
    pc = np.asarray(inputs["previous_context"], dtype=np.float32)
    ctx = np.ascontiguousarray(pc[-10:].reshape(-1))
    assert chunks.shape == (N_TOTAL, 128, 64)
    assert ctx.shape == (S,)

    nc = _get_nc()
    in_maps = [
        {"chunks": chunks[c * N_LOC : (c + 1) * N_LOC], "ctx": ctx}
        for c in range(N_CORES)
    ]
    res = run_bass_kernel_spmd(nc, in_maps, core_ids=list(range(N_CORES)),
                               trace=trace, **kw)
    # out[p, t] = score of local chunk t*128+p -> transpose to chunk order
    full = np.concatenate([r["out"].T.reshape(-1) for r in res.results])
    return full.astype(np.float32), res


def kernel(**inputs) -> np.ndarray:
    return run(inputs)[0]
